# revision 45
# baseline (speedup 1.0000x reference)
"""Distributed Bass kernel for nn_Attention_33354716021494 on 8 TRN2 NeuronCores.

Reference computation (B=2, S=2048, D=1024, H=16, hd=64, f32):
    qkv = x @ w_qkv.T ; split q,k,v ; per-head RoPE on q,k ;
    attn = softmax(mask(q k^T / 8)) ; out = (attn @ v) reshaped @ w_out.T
Sharding: batch x head-group. Core c handles batch b = c//4 and heads
4*(c%4) .. 4*(c%4)+4; host sums the 4 partial out-projections per batch.

Pipeline layout (v4):
  - transposed on-chip layout: Q^T,K^T [hd, S]; scores [k, q]; softmax
    without max-subtraction; rowsums via a ones-column appended to V.
  - the scalar engine (exp) is the phase-2 floor (~93us), so the
    attention streams for q-tiles 0/1 (and qi=2, j=0) are emitted
    INTERLEAVED with the second half of the projection phase: their exp
    work hides under projection PE work that has no scalar dependency.
  - within a k-stream, AV matmuls trail the score matmuls by one group
    and the previous q-tile's output projection is injected into the
    j=1 stream, so exp latency and the softmax-normalize chain always
    have PE work to hide under.
  - PSUM: one pool, tags qk (2x [128,QT] f32, shared by QKV projection,
    V projection and output projection), sc (2x [128,2*QT]), o (2x
    [65,QT]) = 8 banks exactly.
"""

import sys

if "/opt/trn_rl_repo" not in sys.path:
    sys.path.insert(0, "/opt/trn_rl_repo")

import numpy as np
import ml_dtypes

import concourse.bass as bass
import concourse.bacc as bacc
import concourse.tile as tile
import concourse.mybir as mybir
from concourse.bass_utils import run_bass_kernel_spmd

BF16 = mybir.dt.bfloat16
F32 = mybir.dt.float32
NP_BF16 = ml_dtypes.bfloat16

B, S, D, H = 2, 2048, 1024, 16
HD = D // H                      # 64
N_CORES = 8
GROUPS_PER_BATCH = 4             # head groups
HEADS_PER_CORE = H // GROUPS_PER_BATCH   # 4
DL = HEADS_PER_CORE * HD         # 256 local head dims per core
SCALE = HD ** -0.5               # 0.125
NEG = -1.0e9

QT = 512                         # q-tile width (one PSUM bank)
KT = 128                         # k-block height (partition dim)
SLAB = 1024                      # RoPE slab width
USE_PBCAST = False               # gpsimd partition_broadcast: WRONG ON HW
                                 # (sim passes, hardware output garbage)


def build_nc():
    """Build the per-core Bass graph (SPMD: all 8 cores run this graph)."""
    nc = bacc.Bacc(None, target_bir_lowering=False, debug=False,
                   num_devices=N_CORES)

    KC = D // 128                # contraction chunks for the projections
    NQ = S // QT                 # q tiles
    NKB = S // KT                # k blocks
    NSLAB = S // SLAB

    # ---- kernel I/O ----
    xT = nc.declare_dram_parameter("xT", [D, S], BF16, isOutput=False)
    wqT = nc.declare_dram_parameter("wqT", [D, DL], BF16, isOutput=False)
    wkT = nc.declare_dram_parameter("wkT", [D, DL], BF16, isOutput=False)
    wvT = nc.declare_dram_parameter("wvT", [D, DL], BF16, isOutput=False)
    woT = nc.declare_dram_parameter("woT", [DL, D], BF16, isOutput=False)
    cos2 = nc.declare_dram_parameter("cos2", [128, S], BF16, isOutput=False)
    sins = nc.declare_dram_parameter("sins", [128, S], BF16, isOutput=False)
    trim = nc.declare_dram_parameter("trim", [128, 128], BF16, isOutput=False)
    out = nc.declare_dram_parameter("out", [D, S], BF16, isOutput=True)
    rsr_dram = nc.dram_tensor("rsr_dram", [2, 2, 2, QT], F32)

    with tile.TileContext(nc) as tc:
        with tc.tile_pool(name="persist", bufs=1) as pp:
            xt_sb = [pp.tile([128, S], BF16, tag=f"xt{k}", name=f"xt{k}")
                     for k in range(KC)]
            qt_sb = pp.tile([128, 2, S], BF16, tag="qt")
            kt_sb = pp.tile([128, 2, S], BF16, tag="kt")
            v_sb = pp.tile([128, S // 128, 65 * HEADS_PER_CORE], BF16, tag="v")
            ctx_sb = pp.tile([128, 2, S], BF16, tag="ctx")
            cos_sb = pp.tile([128, S], BF16, tag="cos")
            sin_sb = pp.tile([128, S], BF16, tag="sin")
            tri_sb = pp.tile([128, 128], BF16, tag="tri")
            wq_sb = pp.tile([128, KC, DL], BF16, tag="wq")
            wk_sb = pp.tile([128, KC, DL], BF16, tag="wk")
            wv_sb = pp.tile([128, KC, DL], BF16, tag="wv")
            wo_sb = pp.tile([128, DL // 128, D], BF16, tag="wo")
            # ping-pong rowsum tiles (memset once so the unused partition
            # rows always hold 1.0 -> reciprocal stays finite)
            rs_pp = [[pp.tile([128, QT], F32, tag=f"rs{i}{j}",
                               name=f"rs{i}{j}") for j in range(2)]
                     for i in range(2)]
            rsr_pp = [[pp.tile([128, QT], F32, tag=f"rsr{i}{j}",
                               name=f"rsr{i}{j}") for j in range(2)]
                      for i in range(2)]
            scr_pp = [[pp.tile([128, QT], F32, tag=f"scr{i}{j}",
                               name=f"scr{i}{j}") for j in range(2)]
                      for i in range(2)]
            # dedicated, pre-zeroed exp buffers for diagonal blocks, both
            # heads-in-pair interleaved so ONE exp call covers both:
            # [rel offset r][:, h01, :] keeps cols < 128*r permanently 0
            es_diag = [pp.tile([128, 2, QT], BF16, tag=f"esd{r}",
                               name=f"esd{r}")
                       for r in range(QT // KT)]
            # selector + bf16 reciprocal staging for the last tile's
            # PE-matmul rowsum broadcast (replaces the DRAM round-trip)
            sel_sb = pp.tile([64, 128], BF16, tag="sel")
            rsr_bf = pp.tile([64, QT], BF16, tag="rsrb")

            # ---- loads: wv + x interleaved per k-chunk in a narrow first
            # column strip so the first v_proj matmul unblocks after
            # ~100KB of DMA; wk right after so the K slab isn't gated ----
            xTr = xT.ap().rearrange("(c p) s -> c p s", p=128)
            wkTr = wkT.ap().rearrange("(c p) m -> p c m", p=128)
            wqTr = wqT.ap().rearrange("(c p) m -> p c m", p=128)
            for k in range(KC):
                nc.sync.dma_start(
                    wv_sb[:, k, :],
                    wvT.ap().rearrange("(c p) m -> c p m", p=128)[k])
                nc.sync.dma_start(xt_sb[k][:, 0:128], xTr[k][:, 0:128])
            nc.sync.dma_start(wk_sb[:], wkTr)
            for k in range(KC):
                nc.sync.dma_start(xt_sb[k][:, 128:512], xTr[k][:, 128:512])
            for quarter in range(1, 4):
                qsl4 = slice(S // 4 * quarter, S // 4 * (quarter + 1))
                for k in range(KC):
                    nc.sync.dma_start(xt_sb[k][:, qsl4], xTr[k][:, qsl4])
                if quarter == 1:
                    nc.sync.dma_start(wq_sb[:], wqTr)
                    nc.sync.dma_start(cos_sb[:], cos2.ap())
                    nc.sync.dma_start(sin_sb[:], sins.ap())
            nc.sync.dma_start(
                wo_sb[:], woT.ap().rearrange("(c p) m -> p c m", p=128))
            nc.sync.dma_start(tri_sb[:], trim.ap())

            # ones columns of V only (a full-tile memset would stall the
            # early v_proj copies); rowsum/es zero-fills follow on gpsimd
            for hl in range(HEADS_PER_CORE):
                nc.gpsimd.memset(
                    v_sb[:, :, 65 * hl + 64:65 * hl + 65], 1.0)
            for i in range(2):
                for j in range(2):
                    nc.gpsimd.memset(rs_pp[i][j][:], 1.0)
            for r in range(QT // KT):
                nc.gpsimd.memset(es_diag[r][:], 0.0)
            nc.gpsimd.memset(sel_sb[:], 0.0)
            nc.gpsimd.memset(sel_sb[0:1, 0:64], 1.0)
            nc.gpsimd.memset(sel_sb[32:33, 64:128], 1.0)

            with (
                tc.tile_pool(name="p2ps", bufs=1, space="PSUM") as p2ps,
                tc.tile_pool(name="p1sb", bufs=3) as p1sb,
                tc.tile_pool(name="essb", bufs=6) as essb,
                tc.tile_pool(name="otsb", bufs=4) as otsb,
                tc.tile_pool(name="rbsb", bufs=4) as rbsb,
                tc.tile_pool(name="p4sb", bufs=4) as p4sb,
            ):
                # ---------------- phase-1 emitters ----------------
                def v_proj(si):
                    ps = p2ps.tile([128, DL], F32, tag="qk", bufs=2,
                                   name=f"vps{si}")
                    for k in range(KC):
                        nc.tensor.matmul(
                            ps[:],
                            xt_sb[k][:, 128 * si:128 * (si + 1)],
                            wv_sb[:, k, :],
                            start=(k == 0), stop=(k == KC - 1),
                        )
                    nc.any.tensor_copy(
                        v_sb[:, si].rearrange(
                            "p (h c) -> p h c", c=65)[:, :, 0:64],
                        ps.rearrange("p (h c) -> p h c", c=64),
                    )

                def qk_slab_gen(half, dst, wsb, m):
                    """One [128, SLAB] slab of K^T or Q^T with fused RoPE;
                    yields after each qs chunk so attention groups can be
                    interleaved at fine grain."""
                    ssl = slice(SLAB * half, SLAB * (half + 1))
                    rin = p1sb.tile([128, SLAB], BF16, tag="rin")
                    for qs in range(SLAB // QT):
                        ps = p2ps.tile([128, QT], F32, tag="qk", bufs=2,
                                       name=f"qkps{half}{m}{qs}")
                        for k in range(KC):
                            nc.tensor.matmul(
                                ps[:],
                                wsb[:, k, 128 * m:128 * (m + 1)],
                                xt_sb[k][:, SLAB * half + QT * qs:
                                         SLAB * half + QT * (qs + 1)],
                                start=(k == 0), stop=(k == KC - 1),
                            )
                        if half == 0:
                            nc.scalar.copy(
                                rin[:, QT * qs:QT * (qs + 1)], ps[:])
                        else:   # scalar is busy with interleaved exps
                            nc.vector.tensor_copy(
                                rin[:, QT * qs:QT * (qs + 1)], ps[:])
                        yield
                    tmp = p1sb.tile([128, SLAB], BF16, tag="rtmp")
                    for q in range(4):   # partner * sign(sin)
                        src = (q + 1 if q % 2 == 0 else q - 1) * 32
                        nc.vector.tensor_mul(
                            tmp[32 * q:32 * (q + 1), :],
                            rin[src:src + 32, :],
                            sin_sb[src:src + 32, ssl],
                        )
                    qc = p1sb.tile([128, SLAB], BF16, tag="rqc")
                    nc.vector.tensor_mul(qc[:], rin[:], cos_sb[:, ssl])
                    nc.vector.tensor_add(dst[:, m, ssl], qc[:], tmp[:])
                    yield

                # ---------------- phase-2 emitters ----------------
                def emit_out_proj(qi, es=range(D // 128), alt_copy=False):
                    """Output projection for q-tile qi (ctx must be ready)."""
                    qsl = slice(QT * qi, QT * (qi + 1))
                    for e in es:
                        ps = p2ps.tile([128, QT], F32, tag="qk", bufs=2,
                                       name=f"op{qi}_{e}")
                        for kc in range(DL // 128):
                            nc.tensor.matmul(
                                ps[:],
                                wo_sb[:, kc, 128 * e:128 * (e + 1)],
                                ctx_sb[:, kc, qsl],
                                start=(kc == 0), stop=(kc == DL // 128 - 1),
                            )
                        yt = p4sb.tile([128, QT], BF16, tag="ytsb")
                        if alt_copy and e % 2:
                            nc.scalar.copy(yt[:], ps[:])
                        else:
                            nc.vector.tensor_copy(yt[:], ps[:])
                        nc.sync.dma_start(
                            out.ap()[128 * e:128 * (e + 1), qsl], yt[:])

                def qtile_stream(qi):
                    """Generator emitting q-tile qi's attention; yields at
                    group boundaries so the driver can interleave."""
                    qsl = slice(QT * qi, QT * (qi + 1))
                    diag0 = (QT * qi) // KT
                    live = min(NKB, diag0 + QT // KT)
                    groups = []
                    ki = 0
                    while ki < live:
                        nblk = 2 if (ki + 1 < diag0) else 1
                        groups.append((ki, nblk))
                        ki += nblk
                    for j in range(2):
                        o_ps = [p2ps.tile([65, QT], F32, tag="o", bufs=2,
                                          name=f"o{qi}{j}{_h}")
                                for _h in range(2)]

                        def emit_av(ki, nblk, es_mov):
                            for h01 in range(2):
                                hl = 2 * j + h01
                                for t in range(nblk):
                                    kb = ki + t
                                    nc.tensor.matmul(
                                        o_ps[h01][:],
                                        v_sb[:, kb, 65 * hl:65 * hl + 65],
                                        es_mov[h01][t],
                                        start=(kb == 0),
                                        stop=(kb == live - 1),
                                    )

                        pending = None
                        for g, (ki, nblk) in enumerate(groups):
                            if ki >= diag0:      # diagonal single: both
                                # heads share one [128,2,QT] tile so ONE
                                # exp call covers them
                                r = ki - diag0
                                c0 = KT * r
                                spd = p2ps.tile([128, 2, QT], F32, tag="sc",
                                                bufs=2, name=f"sc{qi}{j}{g}")
                                for h01 in range(2):
                                    p0 = 64 * h01
                                    nc.tensor.matmul(
                                        spd[:, h01, :],
                                        kt_sb[p0:p0 + 64, j,
                                              128 * ki:128 * (ki + 1)],
                                        qt_sb[p0:p0 + 64, j, qsl],
                                        start=True, stop=True,
                                    )
                                es2 = es_diag[r]
                                nc.scalar.activation(
                                    es2[:, :, c0:QT], spd[:, :, c0:QT],
                                    mybir.ActivationFunctionType.Exp,
                                    scale=SCALE,
                                )
                                for h01 in range(2):
                                    nc.gpsimd.tensor_mul(
                                        es2[:, h01, c0:c0 + 128],
                                        es2[:, h01, c0:c0 + 128],
                                        tri_sb[:],
                                    )
                                es_mov = [[es2[:, h01, 0:QT]]
                                          for h01 in range(2)]
                            else:
                                sps = [p2ps.tile([128, nblk, QT], F32,
                                                 tag="sc", bufs=2,
                                                 name=f"sc{qi}{j}{g}{_h}")
                                       for _h in range(2)]
                                for t in range(nblk):
                                    kb = ki + t
                                    for h01 in range(2):
                                        p0 = 64 * h01
                                        nc.tensor.matmul(
                                            sps[h01][:, t, :],
                                            kt_sb[p0:p0 + 64, j,
                                                  128 * kb:128 * (kb + 1)],
                                            qt_sb[p0:p0 + 64, j, qsl],
                                            start=True, stop=True,
                                        )
                                es_mov = []
                                for h01 in range(2):
                                    es = essb.tile([128, nblk * QT], BF16,
                                                   tag="es",
                                                   name=f"es{qi}{j}{g}{h01}")
                                    nc.scalar.activation(
                                        es[:], sps[h01].rearrange(
                                            "p a b -> p (a b)"),
                                        mybir.ActivationFunctionType.Exp,
                                        scale=SCALE,
                                    )
                                    es_mov.append(
                                        [es[:, QT * t:QT * (t + 1)]
                                         for t in range(nblk)])
                            if pending is not None:
                                emit_av(*pending)
                            pending = (ki, nblk, es_mov)
                            if qi > 0 and j == 1 and g == 1:
                                emit_out_proj(qi - 1, range(0, 4))
                            if qi > 0 and j == 1 and g == 3:
                                emit_out_proj(qi - 1, range(4, 8))
                            yield
                        emit_av(*pending)

                        # --- normalize; staged so PSUM frees fast; hidden
                        # under whatever the driver emits next ---
                        stage = not (qi == NQ - 1 and j == 1)
                        if stage:
                            ot = otsb.tile([128, QT], F32, tag="ot",
                                           name=f"ot{qi}{j}")
                            for h01 in range(2):
                                nc.vector.tensor_copy(
                                    ot[64 * h01:64 * (h01 + 1), :],
                                    o_ps[h01][0:64, :])
                        rs = rs_pp[qi % 2][j]
                        rsr = rsr_pp[qi % 2][j]
                        scr = scr_pp[qi % 2][j]
                        for h01 in range(2):
                            nc.vector.tensor_copy(
                                rs[32 * h01:32 * h01 + 1, :],
                                o_ps[h01][64:65, :],
                            )
                        if stage:
                            nc.vector.reciprocal_approx_accurate(
                                rsr[:], rs[:], scr[:])
                        else:
                            # last tile: skip the NR refinement on the
                            # exposed tail chain (~1e-3 local error on
                            # 1/8 of the output)
                            nc.vector.reciprocal_approx_fast(rsr[:], rs[:])
                        if stage:
                            for h01 in range(2):
                                nc.sync.dma_start(
                                    rsr_dram.ap()[qi % 2, j, h01],
                                    rsr[32 * h01:32 * h01 + 1, :])
                        if stage:
                            rsb = rbsb.tile([128, QT], F32, tag="rsb",
                                            name=f"rsb{qi}{j}")
                            for h01 in range(2):
                                nc.sync.dma_start(
                                    rsb[64 * h01:64 * h01 + 64, :],
                                    rsr_dram.ap()[qi % 2, j, h01:h01 + 1,
                                                  :].to_broadcast((64, QT)),
                                )
                            nc.vector.tensor_mul(
                                ctx_sb[:, j, qsl], ot[:], rsb[:])
                        else:
                            # last tile: broadcast the reciprocals across
                            # partitions with a bf16 PE selector matmul
                            # instead of the ~5us DRAM round-trip
                            nc.vector.tensor_copy(rsr_bf[:], rsr[0:64, :])
                            rsb_ps = p2ps.tile([128, QT], F32, tag="qk",
                                               bufs=2, name=f"rsbps{qi}{j}")
                            nc.tensor.matmul(
                                rsb_ps[:], sel_sb[:], rsr_bf[:],
                                start=True, stop=True)
                            for h01 in range(2):
                                rsbh = rbsb.tile([64, QT], F32, tag="rsb",
                                                 name=f"rsbh{qi}{j}{h01}")
                                nc.vector.tensor_copy(
                                    rsbh[:],
                                    rsb_ps[64 * h01:64 * (h01 + 1), :])
                                nc.vector.tensor_mul(
                                    ctx_sb[64 * h01:64 * (h01 + 1), j, qsl],
                                    o_ps[h01][0:64, :],
                                    rsbh[:],
                                )
                        yield

                # ---------------- interleaved driver ----------------
                streams = [qtile_stream(qi) for qi in range(NQ)]
                sidx = 0

                def pump(n):
                    nonlocal sidx
                    for _ in range(n):
                        while sidx < NQ:
                            try:
                                next(streams[sidx])
                                break
                            except StopIteration:
                                sidx += 1
                        if sidx >= NQ:
                            return

                for si in range(4):
                    v_proj(si)
                # half 0: projections only (attention needs qt/kt).
                # K slabs m0/m1 interleaved at qs granularity: both qs0
                # chunks need only x cols 0:512 + wk, filling the DMA
                # ramp before quarter 1 lands
                kg = [qk_slab_gen(0, kt_sb, wk_sb, m) for m in range(2)]
                for _ in range(3):
                    for m in range(2):
                        next(kg[m], None)
                for m in range(2):
                    for _ in kg[m]:
                        pass
                for m in range(2):
                    for _ in qk_slab_gen(0, qt_sb, wq_sb, m):
                        pass
                for si in range(4, 8):
                    v_proj(si)
                    pump(1)          # qi=0 starts here
                # half 1: interleave qi0/qi1 attention under the slabs at
                # qs-chunk granularity (short PE pieces between groups)
                for m in range(2):
                    for _ in qk_slab_gen(1, kt_sb, wk_sb, m):
                        pump(1)
                for si in range(8, 10):
                    v_proj(si)
                    pump(1)
                for m in range(2):
                    for _ in qk_slab_gen(1, qt_sb, wq_sb, m):
                        pump(1)
                for si in range(10, 12):
                    v_proj(si)
                    pump(1)
                pump(8)              # finish qi1
                # the last four V slabs are only needed by qi=3's AV, so
                # sprinkle them into qi=2's stream as scalar-free PE cover
                for si in range(12, 16):
                    v_proj(si)
                    pump(2)
                # drain the remaining attention streams
                pump(1000)
                emit_out_proj(NQ - 1, alt_copy=True)

    nc.compile()
    return nc


def host_inputs(x, mask, w_qkv, w_out):
    """Shard + pre-transpose inputs per core. Returns in_maps list."""
    del mask  # causality is baked into the kernel (reference mask is tril)
    inv = 1.0 / (10000.0 ** (np.arange(0, HD, 2, dtype=np.float64) / HD))
    t = np.arange(S, dtype=np.float64)
    fr = np.outer(t, inv)
    emb = np.concatenate([fr, fr], axis=1)          # [S, hd]
    cosT = np.cos(emb).T.astype(np.float32)         # [hd, S]
    sinT = np.sin(emb).T.astype(np.float32)
    cos2 = np.vstack([cosT, cosT]).astype(NP_BF16)
    # value at partition p = sin factor applied to SOURCE partition p
    sins = np.vstack([sinT[32:], -sinT[:32],
                      sinT[32:], -sinT[:32]]).astype(NP_BF16)
    kk = np.arange(128)
    trim = np.where(kk[None, :] >= kk[:, None], 1.0, 0.0).astype(NP_BF16)

    in_maps = []
    for c in range(N_CORES):
        b, g = divmod(c, GROUPS_PER_BATCH)
        rows = slice(DL * g, DL * (g + 1))
        in_maps.append({
            "xT": np.ascontiguousarray(x[b].T).astype(NP_BF16),
            "wqT": np.ascontiguousarray(w_qkv[rows, :].T).astype(NP_BF16),
            "wkT": np.ascontiguousarray(w_qkv[D:][rows, :].T).astype(NP_BF16),
            "wvT": np.ascontiguousarray(w_qkv[2 * D:][rows, :].T).astype(NP_BF16),
            "woT": np.ascontiguousarray(w_out[:, rows].T).astype(NP_BF16),
            "cos2": cos2,
            "sins": sins,
            "trim": trim,
        })
    return in_maps


_NC_CACHE = {}


def _get_nc():
    if "nc" not in _NC_CACHE:
        _NC_CACHE["nc"] = build_nc()
    return _NC_CACHE["nc"]


def _np_reference(x, mask, w_qkv, w_out):
    """Plain numpy fallback (used only if mask is not causal-tril)."""
    q = x @ w_qkv[:D].T
    k = x @ w_qkv[D:2 * D].T
    v = x @ w_qkv[2 * D:].T

    def split(t):
        return t.reshape(B, S, H, HD).transpose(0, 2, 1, 3)

    q, k, v = split(q), split(k), split(v)
    inv = 1.0 / (10000.0 ** (np.arange(0, HD, 2, dtype=np.float64) / HD))
    fr = np.outer(np.arange(S, dtype=np.float64), inv)
    emb = np.concatenate([fr, fr], axis=1)
    cos = np.cos(emb).astype(np.float32)[None, None]
    sin = np.sin(emb).astype(np.float32)[None, None]

    def rot(t):
        return np.concatenate([-t[..., HD // 2:], t[..., :HD // 2]], axis=-1)

    q = q * cos + rot(q) * sin
    k = k * cos + rot(k) * sin
    a = np.einsum("bhqd,bhkd->bhqk", q, k) * SCALE
    a = np.where(mask, a, -np.inf)
    a = a - a.max(axis=-1, keepdims=True)
    a = np.exp(a)
    a /= a.sum(axis=-1, keepdims=True)
    o = np.einsum("bhqk,bhkd->bhqd", a, v)
    o = o.transpose(0, 2, 1, 3).reshape(B, S, D)
    return (o @ w_out.T).astype(np.float32)


def kernel(x, mask, w_qkv, w_out):
    x = np.asarray(x)
    w_qkv = np.asarray(w_qkv)
    w_out = np.asarray(w_out)
    if mask is not None:
        m = np.asarray(mask).reshape(S, S)
        if not np.array_equal(m, np.tril(np.ones((S, S), dtype=bool))):
            return _np_reference(x, m.reshape(1, 1, S, S), w_qkv, w_out)
    nc = _get_nc()
    in_maps = host_inputs(x, mask, w_qkv, w_out)
    res = run_bass_kernel_spmd(nc, in_maps, core_ids=list(range(N_CORES)))
    outs = [r["out"].astype(np.float32) for r in res.results]   # [D, S] each
    y = np.empty((B, S, D), dtype=np.float32)
    for b in range(B):
        yt = sum(outs[GROUPS_PER_BATCH * b + g] for g in range(GROUPS_PER_BATCH))
        y[b] = yt.T
    return y


# revision 46
# speedup vs baseline: 1.1722x; 1.1722x over previous
"""Distributed Bass kernel for nn_Attention_33354716021494 on 8 TRN2 NeuronCores.

Reference computation (B=2, S=2048, D=1024, H=16, hd=64, f32):
    qkv = x @ w_qkv.T ; split q,k,v ; per-head RoPE on q,k ;
    attn = softmax(mask(q k^T / 8)) ; out = (attn @ v) reshaped @ w_out.T
Sharding: batch x head-group. Core c handles batch b = c//4 and heads
4*(c%4) .. 4*(c%4)+4; host sums the 4 partial out-projections per batch.

Pipeline layout (v4):
  - transposed on-chip layout: Q^T,K^T [hd, S]; scores [k, q]; softmax
    without max-subtraction; rowsums via a ones-column appended to V.
  - the scalar engine (exp) is the phase-2 floor (~93us), so the
    attention streams for q-tiles 0/1 (and qi=2, j=0) are emitted
    INTERLEAVED with the second half of the projection phase: their exp
    work hides under projection PE work that has no scalar dependency.
  - within a k-stream, AV matmuls trail the score matmuls by one group
    and the previous q-tile's output projection is injected into the
    j=1 stream, so exp latency and the softmax-normalize chain always
    have PE work to hide under.
  - PSUM: one pool, tags qk (2x [128,QT] f32, shared by QKV projection,
    V projection and output projection), sc (2x [128,2*QT]), o (2x
    [65,QT]) = 8 banks exactly.
"""

import sys

if "/opt/trn_rl_repo" not in sys.path:
    sys.path.insert(0, "/opt/trn_rl_repo")

import numpy as np
import ml_dtypes

import concourse.bass as bass
import concourse.bacc as bacc
import concourse.tile as tile
import concourse.mybir as mybir
from concourse.bass_utils import run_bass_kernel_spmd

BF16 = mybir.dt.bfloat16
F32 = mybir.dt.float32
NP_BF16 = ml_dtypes.bfloat16

B, S, D, H = 2, 2048, 1024, 16
HD = D // H                      # 64
N_CORES = 8
GROUPS_PER_BATCH = 4             # head groups
HEADS_PER_CORE = H // GROUPS_PER_BATCH   # 4
DL = HEADS_PER_CORE * HD         # 256 local head dims per core
SCALE = HD ** -0.5               # 0.125
NEG = -1.0e9

QT = 512                         # q-tile width (one PSUM bank)
KT = 128                         # k-block height (partition dim)
SLAB = 1024                      # RoPE slab width
USE_PBCAST = False               # gpsimd partition_broadcast: WRONG ON HW
                                 # (sim passes, hardware output garbage)


def build_nc():
    """Build the per-core Bass graph (SPMD: all 8 cores run this graph)."""
    nc = bacc.Bacc(None, target_bir_lowering=False, debug=False,
                   num_devices=N_CORES)

    KC = D // 128                # contraction chunks for the projections
    NQ = S // QT                 # q tiles
    NKB = S // KT                # k blocks
    NSLAB = S // SLAB

    # ---- kernel I/O ----
    xT = nc.declare_dram_parameter("xT", [D, S], BF16, isOutput=False)
    wqT = nc.declare_dram_parameter("wqT", [D, DL], BF16, isOutput=False)
    wkT = nc.declare_dram_parameter("wkT", [D, DL], BF16, isOutput=False)
    wvT = nc.declare_dram_parameter("wvT", [D, DL], BF16, isOutput=False)
    woT = nc.declare_dram_parameter("woT", [DL, D], BF16, isOutput=False)
    cos2 = nc.declare_dram_parameter("cos2", [128, S], BF16, isOutput=False)
    sins = nc.declare_dram_parameter("sins", [128, S], BF16, isOutput=False)
    trim = nc.declare_dram_parameter("trim", [128, 128], BF16, isOutput=False)
    out = nc.declare_dram_parameter("out", [D, S], BF16, isOutput=True)
    rsr_dram = nc.dram_tensor("rsr_dram", [2, 2, 2, QT], F32)

    with tile.TileContext(nc) as tc:
        with tc.tile_pool(name="persist", bufs=1) as pp:
            xt_sb = [pp.tile([128, S], BF16, tag=f"xt{k}", name=f"xt{k}")
                     for k in range(KC)]
            qt_sb = pp.tile([128, 2, S], BF16, tag="qt")
            kt_sb = pp.tile([128, 2, S], BF16, tag="kt")
            v_sb = pp.tile([128, S // 128, 65 * HEADS_PER_CORE], BF16, tag="v")
            ctx_sb = pp.tile([128, 2, S], BF16, tag="ctx")
            cos_sb = pp.tile([128, S], BF16, tag="cos")
            sin_sb = pp.tile([128, S], BF16, tag="sin")
            tri_sb = pp.tile([128, 128], BF16, tag="tri")
            wq_sb = pp.tile([128, KC, DL], BF16, tag="wq")
            wk_sb = pp.tile([128, KC, DL], BF16, tag="wk")
            wv_sb = pp.tile([128, KC, DL], BF16, tag="wv")
            wo_sb = pp.tile([128, DL // 128, D], BF16, tag="wo")
            # ping-pong rowsum tiles (memset once so the unused partition
            # rows always hold 1.0 -> reciprocal stays finite)
            rs_pp = [[pp.tile([128, QT], F32, tag=f"rs{i}{j}",
                               name=f"rs{i}{j}") for j in range(2)]
                     for i in range(2)]
            rsr_pp = [[pp.tile([128, QT], F32, tag=f"rsr{i}{j}",
                               name=f"rsr{i}{j}") for j in range(2)]
                      for i in range(2)]
            scr_pp = [[pp.tile([128, QT], F32, tag=f"scr{i}{j}",
                               name=f"scr{i}{j}") for j in range(2)]
                      for i in range(2)]
            # dedicated, pre-zeroed exp buffers for diagonal blocks, both
            # heads-in-pair interleaved so ONE exp call covers both:
            # [rel offset r][:, h01, :] keeps cols < 128*r permanently 0
            es_diag = [pp.tile([128, 2, QT], BF16, tag=f"esd{r}",
                               name=f"esd{r}")
                       for r in range(QT // KT)]

            # ---- loads: wv + x interleaved per k-chunk in a narrow first
            # column strip so the first v_proj matmul unblocks after
            # ~100KB of DMA; wk right after so the K slab isn't gated ----
            xTr = xT.ap().rearrange("(c p) s -> c p s", p=128)
            wkTr = wkT.ap().rearrange("(c p) m -> p c m", p=128)
            wqTr = wqT.ap().rearrange("(c p) m -> p c m", p=128)
            for k in range(KC):
                nc.sync.dma_start(
                    wv_sb[:, k, :],
                    wvT.ap().rearrange("(c p) m -> c p m", p=128)[k])
                nc.sync.dma_start(xt_sb[k][:, 0:128], xTr[k][:, 0:128])
            nc.sync.dma_start(wk_sb[:], wkTr)
            for k in range(KC):
                nc.sync.dma_start(xt_sb[k][:, 128:512], xTr[k][:, 128:512])
            for quarter in range(1, 4):
                qsl4 = slice(S // 4 * quarter, S // 4 * (quarter + 1))
                for k in range(KC):
                    nc.sync.dma_start(xt_sb[k][:, qsl4], xTr[k][:, qsl4])
                if quarter == 1:
                    nc.sync.dma_start(wq_sb[:], wqTr)
                    nc.sync.dma_start(cos_sb[:], cos2.ap())
                    nc.sync.dma_start(sin_sb[:], sins.ap())
            nc.sync.dma_start(
                wo_sb[:], woT.ap().rearrange("(c p) m -> p c m", p=128))
            nc.sync.dma_start(tri_sb[:], trim.ap())

            # ones columns of V only (a full-tile memset would stall the
            # early v_proj copies); rowsum/es zero-fills follow on gpsimd
            for hl in range(HEADS_PER_CORE):
                nc.gpsimd.memset(
                    v_sb[:, :, 65 * hl + 64:65 * hl + 65], 1.0)
            for i in range(2):
                for j in range(2):
                    nc.gpsimd.memset(rs_pp[i][j][:], 1.0)
            for r in range(QT // KT):
                nc.gpsimd.memset(es_diag[r][:], 0.0)

            with (
                tc.tile_pool(name="p2ps", bufs=1, space="PSUM") as p2ps,
                tc.tile_pool(name="p1sb", bufs=3) as p1sb,
                tc.tile_pool(name="essb", bufs=6) as essb,
                tc.tile_pool(name="otsb", bufs=4) as otsb,
                tc.tile_pool(name="rbsb", bufs=4) as rbsb,
                tc.tile_pool(name="p4sb", bufs=4) as p4sb,
            ):
                # ---------------- phase-1 emitters ----------------
                def v_proj(si):
                    ps = p2ps.tile([128, DL], F32, tag="qk", bufs=2,
                                   name=f"vps{si}")
                    for k in range(KC):
                        nc.tensor.matmul(
                            ps[:],
                            xt_sb[k][:, 128 * si:128 * (si + 1)],
                            wv_sb[:, k, :],
                            start=(k == 0), stop=(k == KC - 1),
                        )
                    nc.any.tensor_copy(
                        v_sb[:, si].rearrange(
                            "p (h c) -> p h c", c=65)[:, :, 0:64],
                        ps.rearrange("p (h c) -> p h c", c=64),
                    )

                def qk_slab_gen(half, dst, wsb, m):
                    """One [128, SLAB] slab of K^T or Q^T with fused RoPE;
                    yields after each qs chunk so attention groups can be
                    interleaved at fine grain."""
                    ssl = slice(SLAB * half, SLAB * (half + 1))
                    rin = p1sb.tile([128, SLAB], BF16, tag="rin")
                    for qs in range(SLAB // QT):
                        ps = p2ps.tile([128, QT], F32, tag="qk", bufs=2,
                                       name=f"qkps{half}{m}{qs}")
                        for k in range(KC):
                            nc.tensor.matmul(
                                ps[:],
                                wsb[:, k, 128 * m:128 * (m + 1)],
                                xt_sb[k][:, SLAB * half + QT * qs:
                                         SLAB * half + QT * (qs + 1)],
                                start=(k == 0), stop=(k == KC - 1),
                            )
                        if half == 0:
                            nc.scalar.copy(
                                rin[:, QT * qs:QT * (qs + 1)], ps[:])
                        else:   # scalar is busy with interleaved exps
                            nc.vector.tensor_copy(
                                rin[:, QT * qs:QT * (qs + 1)], ps[:])
                        yield
                    tmp = p1sb.tile([128, SLAB], BF16, tag="rtmp")
                    for q in range(4):   # partner * sign(sin)
                        src = (q + 1 if q % 2 == 0 else q - 1) * 32
                        nc.vector.tensor_mul(
                            tmp[32 * q:32 * (q + 1), :],
                            rin[src:src + 32, :],
                            sin_sb[src:src + 32, ssl],
                        )
                    qc = p1sb.tile([128, SLAB], BF16, tag="rqc")
                    nc.vector.tensor_mul(qc[:], rin[:], cos_sb[:, ssl])
                    nc.vector.tensor_add(dst[:, m, ssl], qc[:], tmp[:])
                    yield

                # ---------------- phase-2 emitters ----------------
                def emit_out_proj(qi, es=range(D // 128), alt_copy=False):
                    """Output projection for q-tile qi (ctx must be ready)."""
                    qsl = slice(QT * qi, QT * (qi + 1))
                    for e in es:
                        ps = p2ps.tile([128, QT], F32, tag="qk", bufs=2,
                                       name=f"op{qi}_{e}")
                        for kc in range(DL // 128):
                            nc.tensor.matmul(
                                ps[:],
                                wo_sb[:, kc, 128 * e:128 * (e + 1)],
                                ctx_sb[:, kc, qsl],
                                start=(kc == 0), stop=(kc == DL // 128 - 1),
                            )
                        yt = p4sb.tile([128, QT], BF16, tag="ytsb")
                        if alt_copy and e % 2:
                            nc.scalar.copy(yt[:], ps[:])
                        else:
                            nc.vector.tensor_copy(yt[:], ps[:])
                        nc.sync.dma_start(
                            out.ap()[128 * e:128 * (e + 1), qsl], yt[:])

                def qtile_stream(qi):
                    """Generator emitting q-tile qi's attention; yields at
                    group boundaries so the driver can interleave."""
                    qsl = slice(QT * qi, QT * (qi + 1))
                    diag0 = (QT * qi) // KT
                    live = min(NKB, diag0 + QT // KT)
                    groups = []
                    ki = 0
                    while ki < live:
                        nblk = 2 if (ki + 1 < diag0) else 1
                        groups.append((ki, nblk))
                        ki += nblk
                    for j in range(2):
                        o_ps = [p2ps.tile([65, QT], F32, tag="o", bufs=2,
                                          name=f"o{qi}{j}{_h}")
                                for _h in range(2)]

                        def emit_av(ki, nblk, es_mov):
                            for h01 in range(2):
                                hl = 2 * j + h01
                                for t in range(nblk):
                                    kb = ki + t
                                    nc.tensor.matmul(
                                        o_ps[h01][:],
                                        v_sb[:, kb, 65 * hl:65 * hl + 65],
                                        es_mov[h01][t],
                                        start=(kb == 0),
                                        stop=(kb == live - 1),
                                    )

                        pending = None
                        for g, (ki, nblk) in enumerate(groups):
                            if ki >= diag0:      # diagonal single: both
                                # heads share one [128,2,QT] tile so ONE
                                # exp call covers them
                                r = ki - diag0
                                c0 = KT * r
                                spd = p2ps.tile([128, 2, QT], F32, tag="sc",
                                                bufs=2, name=f"sc{qi}{j}{g}")
                                for h01 in range(2):
                                    p0 = 64 * h01
                                    nc.tensor.matmul(
                                        spd[:, h01, :],
                                        kt_sb[p0:p0 + 64, j,
                                              128 * ki:128 * (ki + 1)],
                                        qt_sb[p0:p0 + 64, j, qsl],
                                        start=True, stop=True,
                                    )
                                es2 = es_diag[r]
                                nc.scalar.activation(
                                    es2[:, :, c0:QT], spd[:, :, c0:QT],
                                    mybir.ActivationFunctionType.Exp,
                                    scale=SCALE,
                                )
                                for h01 in range(2):
                                    nc.gpsimd.tensor_mul(
                                        es2[:, h01, c0:c0 + 128],
                                        es2[:, h01, c0:c0 + 128],
                                        tri_sb[:],
                                    )
                                es_mov = [[es2[:, h01, 0:QT]]
                                          for h01 in range(2)]
                            else:
                                sps = [p2ps.tile([128, nblk, QT], F32,
                                                 tag="sc", bufs=2,
                                                 name=f"sc{qi}{j}{g}{_h}")
                                       for _h in range(2)]
                                for t in range(nblk):
                                    kb = ki + t
                                    for h01 in range(2):
                                        p0 = 64 * h01
                                        nc.tensor.matmul(
                                            sps[h01][:, t, :],
                                            kt_sb[p0:p0 + 64, j,
                                                  128 * kb:128 * (kb + 1)],
                                            qt_sb[p0:p0 + 64, j, qsl],
                                            start=True, stop=True,
                                        )
                                es_mov = []
                                for h01 in range(2):
                                    es = essb.tile([128, nblk * QT], BF16,
                                                   tag="es",
                                                   name=f"es{qi}{j}{g}{h01}")
                                    nc.scalar.activation(
                                        es[:], sps[h01].rearrange(
                                            "p a b -> p (a b)"),
                                        mybir.ActivationFunctionType.Exp,
                                        scale=SCALE,
                                    )
                                    es_mov.append(
                                        [es[:, QT * t:QT * (t + 1)]
                                         for t in range(nblk)])
                            if pending is not None:
                                emit_av(*pending)
                            pending = (ki, nblk, es_mov)
                            if qi > 0 and j == 1 and g == 1:
                                emit_out_proj(qi - 1, range(0, 4))
                            if qi > 0 and j == 1 and g == 3:
                                emit_out_proj(qi - 1, range(4, 8))
                            yield
                        emit_av(*pending)

                        # --- normalize; staged so PSUM frees fast; hidden
                        # under whatever the driver emits next ---
                        stage = not (qi == NQ - 1 and j == 1)
                        if stage:
                            ot = otsb.tile([128, QT], F32, tag="ot",
                                           name=f"ot{qi}{j}")
                            for h01 in range(2):
                                nc.vector.tensor_copy(
                                    ot[64 * h01:64 * (h01 + 1), :],
                                    o_ps[h01][0:64, :])
                        rs = rs_pp[qi % 2][j]
                        rsr = rsr_pp[qi % 2][j]
                        scr = scr_pp[qi % 2][j]
                        for h01 in range(2):
                            nc.vector.tensor_copy(
                                rs[32 * h01:32 * h01 + 1, :],
                                o_ps[h01][64:65, :],
                            )
                        if stage:
                            nc.vector.reciprocal_approx_accurate(
                                rsr[:], rs[:], scr[:])
                        else:
                            # last tile: skip the NR refinement on the
                            # exposed tail chain (~1e-3 local error on
                            # 1/8 of the output)
                            nc.vector.reciprocal_approx_fast(rsr[:], rs[:])
                        if stage or not USE_PBCAST:
                            for h01 in range(2):
                                nc.sync.dma_start(
                                    rsr_dram.ap()[qi % 2, j, h01],
                                    rsr[32 * h01:32 * h01 + 1, :])
                        if stage:
                            rsb = rbsb.tile([128, QT], F32, tag="rsb",
                                            name=f"rsb{qi}{j}")
                            for h01 in range(2):
                                nc.sync.dma_start(
                                    rsb[64 * h01:64 * h01 + 64, :],
                                    rsr_dram.ap()[qi % 2, j, h01:h01 + 1,
                                                  :].to_broadcast((64, QT)),
                                )
                            nc.vector.tensor_mul(
                                ctx_sb[:, j, qsl], ot[:], rsb[:])
                        else:
                            for h01 in range(2):
                                rsbh = rbsb.tile([64, QT], F32, tag="rsb",
                                                 name=f"rsbh{qi}{j}{h01}")
                                if USE_PBCAST:
                                    nc.gpsimd.partition_broadcast(
                                        rsbh[:],
                                        rsr[32 * h01:32 * h01 + 1, :])
                                else:
                                    nc.sync.dma_start(
                                        rsbh[:],
                                        rsr_dram.ap()[qi % 2, j,
                                                      h01:h01 + 1, :]
                                        .to_broadcast((64, QT)),
                                    )
                                nc.vector.tensor_mul(
                                    ctx_sb[64 * h01:64 * (h01 + 1), j, qsl],
                                    o_ps[h01][0:64, :],
                                    rsbh[:],
                                )
                        yield

                # ---------------- interleaved driver ----------------
                streams = [qtile_stream(qi) for qi in range(NQ)]
                sidx = 0

                def pump(n):
                    nonlocal sidx
                    for _ in range(n):
                        while sidx < NQ:
                            try:
                                next(streams[sidx])
                                break
                            except StopIteration:
                                sidx += 1
                        if sidx >= NQ:
                            return

                for si in range(4):
                    v_proj(si)
                # half 0: projections only (attention needs qt/kt).
                # K slabs m0/m1 interleaved at qs granularity: both qs0
                # chunks need only x cols 0:512 + wk, filling the DMA
                # ramp before quarter 1 lands
                kg = [qk_slab_gen(0, kt_sb, wk_sb, m) for m in range(2)]
                for _ in range(3):
                    for m in range(2):
                        next(kg[m], None)
                for m in range(2):
                    for _ in kg[m]:
                        pass
                for m in range(2):
                    for _ in qk_slab_gen(0, qt_sb, wq_sb, m):
                        pass
                for si in range(4, 8):
                    v_proj(si)
                    pump(1)          # qi=0 starts here
                # half 1: interleave qi0/qi1 attention under the slabs at
                # qs-chunk granularity (short PE pieces between groups)
                for m in range(2):
                    for _ in qk_slab_gen(1, kt_sb, wk_sb, m):
                        pump(1)
                for si in range(8, 10):
                    v_proj(si)
                    pump(1)
                for m in range(2):
                    for _ in qk_slab_gen(1, qt_sb, wq_sb, m):
                        pump(1)
                for si in range(10, 12):
                    v_proj(si)
                    pump(1)
                pump(8)              # finish qi1
                # the last four V slabs are only needed by qi=3's AV, so
                # sprinkle them into qi=2's stream as scalar-free PE cover
                for si in range(12, 16):
                    v_proj(si)
                    pump(2)
                # drain the remaining attention streams
                pump(1000)
                emit_out_proj(NQ - 1, alt_copy=True)

    nc.compile()
    return nc


def host_inputs(x, mask, w_qkv, w_out):
    """Shard + pre-transpose inputs per core. Returns in_maps list."""
    del mask  # causality is baked into the kernel (reference mask is tril)
    inv = 1.0 / (10000.0 ** (np.arange(0, HD, 2, dtype=np.float64) / HD))
    t = np.arange(S, dtype=np.float64)
    fr = np.outer(t, inv)
    emb = np.concatenate([fr, fr], axis=1)          # [S, hd]
    cosT = np.cos(emb).T.astype(np.float32)         # [hd, S]
    sinT = np.sin(emb).T.astype(np.float32)
    cos2 = np.vstack([cosT, cosT]).astype(NP_BF16)
    # value at partition p = sin factor applied to SOURCE partition p
    sins = np.vstack([sinT[32:], -sinT[:32],
                      sinT[32:], -sinT[:32]]).astype(NP_BF16)
    kk = np.arange(128)
    trim = np.where(kk[None, :] >= kk[:, None], 1.0, 0.0).astype(NP_BF16)

    in_maps = []
    for c in range(N_CORES):
        b, g = divmod(c, GROUPS_PER_BATCH)
        rows = slice(DL * g, DL * (g + 1))
        in_maps.append({
            "xT": np.ascontiguousarray(x[b].T).astype(NP_BF16),
            "wqT": np.ascontiguousarray(w_qkv[rows, :].T).astype(NP_BF16),
            "wkT": np.ascontiguousarray(w_qkv[D:][rows, :].T).astype(NP_BF16),
            "wvT": np.ascontiguousarray(w_qkv[2 * D:][rows, :].T).astype(NP_BF16),
            "woT": np.ascontiguousarray(w_out[:, rows].T).astype(NP_BF16),
            "cos2": cos2,
            "sins": sins,
            "trim": trim,
        })
    return in_maps


_NC_CACHE = {}


def _get_nc():
    if "nc" not in _NC_CACHE:
        _NC_CACHE["nc"] = build_nc()
    return _NC_CACHE["nc"]


def _np_reference(x, mask, w_qkv, w_out):
    """Plain numpy fallback (used only if mask is not causal-tril)."""
    q = x @ w_qkv[:D].T
    k = x @ w_qkv[D:2 * D].T
    v = x @ w_qkv[2 * D:].T

    def split(t):
        return t.reshape(B, S, H, HD).transpose(0, 2, 1, 3)

    q, k, v = split(q), split(k), split(v)
    inv = 1.0 / (10000.0 ** (np.arange(0, HD, 2, dtype=np.float64) / HD))
    fr = np.outer(np.arange(S, dtype=np.float64), inv)
    emb = np.concatenate([fr, fr], axis=1)
    cos = np.cos(emb).astype(np.float32)[None, None]
    sin = np.sin(emb).astype(np.float32)[None, None]

    def rot(t):
        return np.concatenate([-t[..., HD // 2:], t[..., :HD // 2]], axis=-1)

    q = q * cos + rot(q) * sin
    k = k * cos + rot(k) * sin
    a = np.einsum("bhqd,bhkd->bhqk", q, k) * SCALE
    a = np.where(mask, a, -np.inf)
    a = a - a.max(axis=-1, keepdims=True)
    a = np.exp(a)
    a /= a.sum(axis=-1, keepdims=True)
    o = np.einsum("bhqk,bhkd->bhqd", a, v)
    o = o.transpose(0, 2, 1, 3).reshape(B, S, D)
    return (o @ w_out.T).astype(np.float32)


def kernel(x, mask, w_qkv, w_out):
    x = np.asarray(x)
    w_qkv = np.asarray(w_qkv)
    w_out = np.asarray(w_out)
    if mask is not None:
        m = np.asarray(mask).reshape(S, S)
        if not np.array_equal(m, np.tril(np.ones((S, S), dtype=bool))):
            return _np_reference(x, m.reshape(1, 1, S, S), w_qkv, w_out)
    nc = _get_nc()
    in_maps = host_inputs(x, mask, w_qkv, w_out)
    res = run_bass_kernel_spmd(nc, in_maps, core_ids=list(range(N_CORES)))
    outs = [r["out"].astype(np.float32) for r in res.results]   # [D, S] each
    y = np.empty((B, S, D), dtype=np.float32)
    for b in range(B):
        yt = sum(outs[GROUPS_PER_BATCH * b + g] for g in range(GROUPS_PER_BATCH))
        y[b] = yt.T
    return y
